# revision 27
# baseline (speedup 1.0000x reference)
"""DeepSeekMoE Trainium2 kernel (8 NeuronCores, expert-parallel + host dispatch).

Strategy
--------
The reference computes every expert densely on all T=4096 tokens and then
zero-weights unrouted (token, expert) pairs.  Only top-2 of 8 experts have
nonzero weight, so ~3/4 of that expert compute is wasted.  This kernel moves
the routing decision to the host and runs expert-parallel:

  host:   router logits / softmax / top-2 / renormalize — computed with the
          exact same jax CPU ops as the reference so tie-breaks match
          bit-for-bit (the min 2nd/3rd logit gap is ~2e-6; a mis-routed token
          would blow the error budget).  Tokens are gathered per expert and
          padded to the perfectly balanced capacity C = T*K/E = 1024; the few
          overflow assignments beyond C (load-imbalance tail, ~1% of pairs)
          are computed exactly on the host.  All device operands are packed
          on the host into the exact [128-partition, ...] SBUF layouts so
          every DMA is a contiguous 128-row slab (descriptor issue on the
          Sync engine costs ~600ns+ per pattern row otherwise).
  core e: shared-expert pass over its 512-token shard, plus expert e's pass
          over its C gathered tokens: hT = gelu(w1.T @ xT + b1) (b1 applied
          free via the per-partition activation-bias port), out = hT.T @ w2.
          All matmul operands are bf16 (full PE rate, half the DMA/SBUF of
          fp32r), accumulation fp32 in PSUM, outputs fp32.  mm2 runs
          m-tile-outer so each m-tile's PSUM bank is evicted (DVE+ACT in
          parallel) while the next m-tile's matmuls run.
  host:   out = shared + sum of top-2 weighted gathered expert rows (exact
          fp32 scatter-add; b2/router_b contributions added exactly here).

Per-core compute is (512 + 1024) token-passes instead of the dense
baseline's 9*512 = 4608 — exactly 3x fewer PE cycles, and all cores are
identical so SPMD padding also balances the instruction streams.
"""

import sys

sys.path.insert(0, "/opt/trn_rl_repo")

from contextlib import ExitStack

import ml_dtypes
import numpy as np

import concourse.bass as bass  # noqa: F401  (engine types resolve through bacc)
import concourse.tile as tile
from concourse import bacc, mybir
from concourse.bass_utils import run_bass_kernel_spmd

F32 = mybir.dt.float32
BF16 = mybir.dt.bfloat16
AF = mybir.ActivationFunctionType
BF = ml_dtypes.bfloat16

D, H, E = 1024, 2048, 8
B, S = 2, 2048
T = B * S
TOP_K = 2
NCORES = 8
SC = T // NCORES          # 512 shared-expert tokens per core
KD = D // 128             # 8 k-tiles over D
KH = H // 128             # 16 k-tiles over H
NQ = 4                    # hid quarters for mm1 psum


def _gelu_exact(z):
    try:
        from scipy.special import erf

        return 0.5 * z * (1.0 + erf(z / np.float32(np.sqrt(2.0))))
    except Exception:
        import math

        ef = np.vectorize(math.erf, otypes=[np.float32])
        return 0.5 * z * (1.0 + ef(z / np.float32(np.sqrt(2.0))))


def _pack_xT(xrows: np.ndarray, width: int) -> np.ndarray:
    """[n, D] bf16 tokens -> [128, KD, width] slab (xT tiles), zero padded."""
    n = xrows.shape[0]
    out = np.zeros((128, KD, width), BF)
    out[:, :, :n] = xrows.reshape(n, KD, 128).transpose(2, 1, 0)
    return out


def _pack_w1(w: np.ndarray) -> np.ndarray:
    """[D, H] -> [NQ, 128, KD, 512] per-quarter contiguous lhsT slabs."""
    return np.ascontiguousarray(
        w.reshape(KD, 128, NQ, 512).transpose(2, 1, 0, 3).astype(BF)
    )


def _pack_w2(w: np.ndarray) -> np.ndarray:
    """[H, D] -> [128, KH, D] contiguous rhs slab."""
    return np.ascontiguousarray(w.reshape(KH, 128, D).transpose(1, 0, 2).astype(BF))


def build_program(C: int):
    nc = bacc.Bacc("TRN2", debug=False)

    xsT = nc.dram_tensor("xsT", [128, KD, SC], BF16, kind="ExternalInput").ap()
    xgT = nc.dram_tensor("xgT", [128, KD, C], BF16, kind="ExternalInput").ap()
    sw1 = nc.dram_tensor("sw1", [NQ, 128, KD, 512], BF16, kind="ExternalInput").ap()
    sw2 = nc.dram_tensor("sw2", [128, KH, D], BF16, kind="ExternalInput").ap()
    w1 = nc.dram_tensor("w1", [NQ, 128, KD, 512], BF16, kind="ExternalInput").ap()
    w2 = nc.dram_tensor("w2", [128, KH, D], BF16, kind="ExternalInput").ap()
    b1s = nc.dram_tensor("b1s", [128, KH], F32, kind="ExternalInput").ap()
    b1e = nc.dram_tensor("b1e", [128, KH], F32, kind="ExternalInput").ap()
    outs = nc.dram_tensor("outs", [SC, D], F32, kind="ExternalOutput").ap()
    outg = nc.dram_tensor("outg", [C, D], F32, kind="ExternalOutput").ap()

    chunks = []
    c0 = 0
    while c0 < C:
        w = min(512, C - c0)
        chunks.append((c0, w))
        c0 += w

    with tile.TileContext(nc) as tc, ExitStack() as ctx:
        resp = ctx.enter_context(tc.tile_pool(name="resp", bufs=1))
        psp = ctx.enter_context(tc.tile_pool(name="psp", bufs=8, space="PSUM"))
        htp = resp
        otp = resp

        # Every DMA below is a contiguous [128, ...] slab (one descriptor
        # row per partition).  Only the first mm1 quarter's operands (split in
        # halves, ~1MB critical) are issued before the first matmuls; the bulk
        # loads are interleaved between mm1-quarter emissions via post_q so
        # they stream under compute instead of competing with the critical
        # pair for HBM bandwidth at t=0.
        sw1q = []
        sw1q.append(resp.tile([128, KD, 512], BF16, tag="sw1q0", name="sw1q0"))
        xsT_sb = resp.tile([128, KD, SC], BF16, tag="xsT")
        # first matmul needs only the k=0 slices (128KB each; per-queue DMA is
        # ~100-170GB/s with ~1.5us startup, so small first pieces matter)
        nc.sync.dma_start(out=sw1q[0][:, 0, :], in_=sw1[0][:, 0, :])
        nc.sync.dma_start(out=xsT_sb[:, 0, :], in_=xsT[:, 0, :])
        nc.sync.dma_start(out=sw1q[0][:, 1, :], in_=sw1[0][:, 1, :])
        nc.sync.dma_start(out=xsT_sb[:, 1, :], in_=xsT[:, 1, :])
        nc.sync.dma_start(out=sw1q[0][:, 2:4, :], in_=sw1[0][:, 2:4, :])
        nc.sync.dma_start(out=xsT_sb[:, 2:4, :], in_=xsT[:, 2:4, :])
        b1s_sb = resp.tile([128, KH], F32, tag="b1s")
        nc.sync.dma_start(out=b1s_sb, in_=b1s)
        nc.sync.dma_start(out=sw1q[0][:, 4:KD, :], in_=sw1[0][:, 4:KD, :])
        nc.sync.dma_start(out=xsT_sb[:, 4:KD, :], in_=xsT[:, 4:KD, :])
        for q in range(1, NQ):
            t = resp.tile([128, KD, 512], BF16, tag=f"sw1q{q}", name=f"sw1q{q}")
            sw1q.append(t)
        sw2_sb = resp.tile([128, KH, D], BF16, tag="sw2")
        xgT_sb = resp.tile([128, KD, C], BF16, tag="xgT")
        b1e_sb = resp.tile([128, KH], F32, tag="b1e")
        ew1q = [
            resp.tile([128, KD, 512], BF16, tag=f"ew1q{q}", name=f"ew1q_{q}")
            for q in range(NQ)
        ]
        ew2_sb = resp.tile([128, KH, D], BF16, tag="ew2")

        def _load_bulk():
            nc.sync.dma_start(out=sw2_sb, in_=sw2)
            nc.sync.dma_start(out=xgT_sb, in_=xgT)
            nc.sync.dma_start(out=b1e_sb, in_=b1e)
            for q in range(NQ):
                nc.sync.dma_start(out=ew1q[q], in_=w1[q])
            nc.sync.dma_start(out=ew2_sb, in_=w2)

        shared_post_q = {
            0: lambda: nc.sync.dma_start(out=sw1q[1], in_=sw1[1]),
            1: lambda: nc.sync.dma_start(out=sw1q[2], in_=sw1[2]),
            2: lambda: (nc.sync.dma_start(out=sw1q[3], in_=sw1[3]), _load_bulk()),
        }

        def emit_pass(xT_sb, c0, W, m_base, outdram, w1q, w2_sb, b1_sb, pi,
                      post_q=None, last=False):
            MTc = W // 128
            # mm1: hT[j] = gelu(w1.T @ xT + b1) in hid quarters of 4 psum banks
            hts = []
            for q in range(NQ):
                phs = [
                    psp.tile([128, W], F32, tag="ps", name=f"ph{pi}_{q}_{mh}")
                    for mh in range(4)
                ]
                for k in range(KD):
                    for mh in range(4):
                        nc.tensor.matmul(
                            phs[mh],
                            w1q[q][:, k, mh * 128 : (mh + 1) * 128],
                            xT_sb[:, k, c0 : c0 + W],
                            start=(k == 0),
                            stop=(k == KD - 1),
                        )
                if post_q and q in post_q:
                    post_q[q]()
                for mh in range(4):
                    j = q * 4 + mh
                    ht = htp.tile([128, 512], BF16, tag=f"ht{j}", name=f"ht{pi}_{j}")
                    nc.scalar.activation(
                        ht[:, :W], phs[mh][:], AF.Gelu, bias=b1_sb[:, j : j + 1]
                    )
                    hts.append(ht)

            # mm2: out[mt] = sum_k hT[k][:, mt].T @ w2[k]; m-tile-outer so each
            # m-tile evicts (DVE || ACT copy halves) under the next one's MMs.
            ov = outdram.rearrange("(m p) d -> p m d", p=128)
            for mt in range(MTc):
                pon = [
                    psp.tile([128, 512], F32, tag="ps", name=f"po{pi}_{mt}_{n}")
                    for n in range(2)
                ]
                for k in range(KH):
                    for n in range(2):
                        nc.tensor.matmul(
                            pon[n],
                            hts[k][:, mt * 128 : (mt + 1) * 128],
                            w2_sb[:, k, n * 512 : (n + 1) * 512],
                            start=(k == 0),
                            stop=(k == KH - 1),
                        )
                ot = otp.tile([128, D], F32, tag="ot", bufs=4, name=f"ot{pi}_{mt}")
                nc.vector.tensor_copy(ot[:, 0:512], pon[0][:])
                if last and mt == MTc - 1:
                    # final m-tile: start the n0-half store while the ACT copy
                    # of the n1-half is still draining, shortening the tail
                    nc.sync.dma_start(
                        out=ov[:, m_base + mt, 0:512], in_=ot[:, 0:512]
                    )
                    nc.scalar.copy(ot[:, 512:1024], pon[1][:])
                    nc.sync.dma_start(
                        out=ov[:, m_base + mt, 512:1024], in_=ot[:, 512:1024]
                    )
                else:
                    nc.scalar.copy(ot[:, 512:1024], pon[1][:])
                    nc.sync.dma_start(out=ov[:, m_base + mt, :], in_=ot)

        emit_pass(xsT_sb, 0, SC, 0, outs, sw1q, sw2_sb, b1s_sb, 0,
                  post_q=shared_post_q)
        for ci, (c0, w) in enumerate(chunks):
            emit_pass(xgT_sb, c0, w, c0 // 128, outg, ew1q, ew2_sb, b1e_sb,
                      1 + ci, last=(ci == len(chunks) - 1))

    nc.compile()
    return nc


_programs: dict = {}
LAST_RESULTS = None


def _get_program(C: int):
    if C not in _programs:
        _programs[C] = build_program(C)
    return _programs[C]


def _route_jax(flat, router_w, router_b):
    """Replicate reference router bit-for-bit (same jax CPU ops)."""
    import jax
    import jax.numpy as jnp

    cpu = jax.devices("cpu")[0]
    with jax.default_device(cpu):
        probs = jax.nn.softmax(
            jnp.asarray(flat) @ jnp.asarray(router_w) + jnp.asarray(router_b), axis=-1
        )
        top_w, top_i = jax.lax.top_k(probs, TOP_K)
        top_w = top_w / jnp.sum(top_w, axis=-1, keepdims=True)
        return np.asarray(top_w), np.asarray(top_i)


def _route_np(flat, router_w, router_b):
    logits = (
        flat.astype(np.float64) @ router_w.astype(np.float64)
        + router_b.astype(np.float64)
    )
    ar = np.arange(T)
    i1 = np.argmax(logits, 1)
    l1 = logits[ar, i1]
    lm = logits.copy()
    lm[ar, i1] = -np.inf
    i2 = np.argmax(lm, 1)
    l2 = lm[ar, i2]
    wa = 1.0 / (1.0 + np.exp(l2 - l1))
    top_w = np.stack([wa, 1.0 - wa], 1).astype(np.float32)
    top_i = np.stack([i1, i2], 1).astype(np.int32)
    return top_w, top_i


def kernel(x, router_w, router_b, sw1, sb1, sw2, sb2, ew1, eb1, ew2, eb2):
    global LAST_RESULTS
    x = np.asarray(x, np.float32)
    flat = np.ascontiguousarray(x.reshape(T, D))
    rw = np.ascontiguousarray(np.asarray(router_w, np.float32))
    rb = np.asarray(router_b, np.float32).reshape(E)
    try:
        top_w, top_i = _route_jax(flat, rw, rb)
    except Exception:
        top_w, top_i = _route_np(flat, rw, rb)
    i1 = top_i[:, 0].astype(np.int64)
    i2 = top_i[:, 1].astype(np.int64)

    rows_l, wgt_l = [], []
    for e in range(E):
        sel1 = i1 == e
        rows = np.nonzero(sel1 | (i2 == e))[0]
        wgt = np.where(sel1[rows], top_w[rows, 0], top_w[rows, 1]).astype(np.float32)
        rows_l.append(rows)
        wgt_l.append(wgt)
    maxc = max(len(r) for r in rows_l)
    # Device capacity is the perfectly balanced T*K/E; the few overflow
    # assignments beyond it (load imbalance tail) are computed exactly on the
    # host, keeping every core's padded pass the same minimal size.
    CAP = T * TOP_K // E
    C = max(128, min(-(-maxc // 128) * 128, CAP))

    nc = _get_program(C)

    xq = flat.astype(BF)
    sw1p = _pack_w1(np.asarray(sw1, np.float32))
    sw2p = _pack_w2(np.asarray(sw2, np.float32))
    ew1f = np.asarray(ew1, np.float32)
    ew2f = np.asarray(ew2, np.float32)
    b1s_arr = np.ascontiguousarray(np.asarray(sb1, np.float32).reshape(KH, 128).T)
    eb1f = np.asarray(eb1, np.float32)

    in_maps = []
    for c in range(NCORES):
        rows = rows_l[c][:C]
        in_maps.append(
            {
                "xsT": _pack_xT(xq[c * SC : (c + 1) * SC], SC),
                "xgT": _pack_xT(xq[rows], C),
                "sw1": sw1p,
                "sw2": sw2p,
                "w1": _pack_w1(ew1f[c]),
                "w2": _pack_w2(ew2f[c]),
                "b1s": b1s_arr,
                "b1e": np.ascontiguousarray(eb1f[c].reshape(KH, 128).T),
            }
        )

    def _spot_check(res) -> bool:
        """Recompute a few shared-expert rows on host; catches a wedged
        device returning stale/garbage buffers (seen after NRT errors)."""
        sw1f = np.asarray(sw1, np.float32)
        sw2f = np.asarray(sw2, np.float32)
        sb1f = np.asarray(sb1, np.float32).reshape(H)
        for c in (0, NCORES - 1):
            got = res.results[c]["outs"]
            if not np.all(np.isfinite(got)):
                return False
            t = c * SC + 7
            ref = _gelu_exact(flat[t] @ sw1f + sb1f) @ sw2f
            scale = max(1.0, float(np.abs(ref).max()))
            if float(np.abs(got[7] - ref).max()) > 1e-2 * scale:
                return False
        return True

    res = None
    for attempt in range(4):
        try:
            res = run_bass_kernel_spmd(nc, in_maps, core_ids=list(range(NCORES)))
            if _spot_check(res):
                break
            res = None
        except Exception:
            if attempt == 3:
                raise
        import time as _time

        _time.sleep(5)  # transient device errors recover on retry
    if res is None:
        raise RuntimeError("device returned inconsistent results on all attempts")
    LAST_RESULTS = res

    out = np.ascontiguousarray(
        np.concatenate([res.results[c]["outs"] for c in range(NCORES)], axis=0),
        dtype=np.float32,
    )
    for e in range(E):
        rows = rows_l[e][:C]
        if len(rows):
            out[rows] += (
                wgt_l[e][: len(rows), None] * res.results[e]["outg"][: len(rows)]
            )
        over = rows_l[e][C:]
        if len(over):
            # exact fp32 host compute for capacity-overflow assignments
            z = flat[over] @ np.asarray(ew1[e], np.float32) + eb1f[e]
            y = _gelu_exact(z) @ np.asarray(ew2[e], np.float32)
            out[over] += wgt_l[e][C:, None] * y

    sb2f = np.asarray(sb2, np.float32).reshape(D)
    if sb2f.any():
        out += sb2f[None, :]
    eb2f = np.asarray(eb2, np.float32)
    if eb2f.any():
        comb = np.zeros((T, E), np.float32)
        comb[np.arange(T), i1] = top_w[:, 0]
        comb[np.arange(T), i2] = top_w[:, 1]
        out += comb @ eb2f
    return out.reshape(B, S, D)


# revision 28
# speedup vs baseline: 1.0137x; 1.0137x over previous
"""DeepSeekMoE Trainium2 kernel (8 NeuronCores, expert-parallel + host dispatch).

Strategy
--------
The reference computes every expert densely on all T=4096 tokens and then
zero-weights unrouted (token, expert) pairs.  Only top-2 of 8 experts have
nonzero weight, so ~3/4 of that expert compute is wasted.  This kernel moves
the routing decision to the host and runs expert-parallel:

  host:   router logits / softmax / top-2 / renormalize — computed with the
          exact same jax CPU ops as the reference so tie-breaks match
          bit-for-bit (the min 2nd/3rd logit gap is ~2e-6; a mis-routed token
          would blow the error budget).  Tokens are gathered per expert and
          padded to the perfectly balanced capacity C = T*K/E = 1024; the few
          overflow assignments beyond C (load-imbalance tail, ~1% of pairs)
          are computed exactly on the host.  All device operands are packed
          on the host into the exact [128-partition, ...] SBUF layouts so
          every DMA is a contiguous 128-row slab (descriptor issue on the
          Sync engine costs ~600ns+ per pattern row otherwise).
  core e: shared-expert pass over its 512-token shard, plus expert e's pass
          over its C gathered tokens: hT = gelu(w1.T @ xT + b1) (b1 applied
          free via the per-partition activation-bias port), out = hT.T @ w2.
          All matmul operands are bf16 (full PE rate, half the DMA/SBUF of
          fp32r), accumulation fp32 in PSUM, outputs fp32.  mm2 runs
          m-tile-outer so each m-tile's PSUM bank is evicted (DVE+ACT in
          parallel) while the next m-tile's matmuls run.
  host:   out = shared + sum of top-2 weighted gathered expert rows (exact
          fp32 scatter-add; b2/router_b contributions added exactly here).

Per-core compute is (512 + 1024) token-passes instead of the dense
baseline's 9*512 = 4608 — exactly 3x fewer PE cycles, and all cores are
identical so SPMD padding also balances the instruction streams.
"""

import sys

sys.path.insert(0, "/opt/trn_rl_repo")

from contextlib import ExitStack

import ml_dtypes
import numpy as np

import concourse.bass as bass  # noqa: F401  (engine types resolve through bacc)
import concourse.tile as tile
from concourse import bacc, mybir
from concourse.bass_utils import run_bass_kernel_spmd

F32 = mybir.dt.float32
BF16 = mybir.dt.bfloat16
AF = mybir.ActivationFunctionType
BF = ml_dtypes.bfloat16

D, H, E = 1024, 2048, 8
B, S = 2, 2048
T = B * S
TOP_K = 2
NCORES = 8
SC = T // NCORES          # 512 shared-expert tokens per core
KD = D // 128             # 8 k-tiles over D
KH = H // 128             # 16 k-tiles over H
NQ = 4                    # hid quarters for mm1 psum


def _gelu_exact(z):
    try:
        from scipy.special import erf

        return 0.5 * z * (1.0 + erf(z / np.float32(np.sqrt(2.0))))
    except Exception:
        import math

        ef = np.vectorize(math.erf, otypes=[np.float32])
        return 0.5 * z * (1.0 + ef(z / np.float32(np.sqrt(2.0))))


def _pack_xT(xrows: np.ndarray, width: int) -> np.ndarray:
    """[n, D] bf16 tokens -> [128, KD, width] slab (xT tiles), zero padded."""
    n = xrows.shape[0]
    out = np.zeros((128, KD, width), BF)
    out[:, :, :n] = xrows.reshape(n, KD, 128).transpose(2, 1, 0)
    return out


def _pack_w1(w: np.ndarray) -> np.ndarray:
    """[D, H] -> [NQ, 128, KD, 512] per-quarter contiguous lhsT slabs."""
    return np.ascontiguousarray(
        w.reshape(KD, 128, NQ, 512).transpose(2, 1, 0, 3).astype(BF)
    )


def _pack_w2(w: np.ndarray) -> np.ndarray:
    """[H, D] -> [128, KH, D] contiguous rhs slab."""
    return np.ascontiguousarray(w.reshape(KH, 128, D).transpose(1, 0, 2).astype(BF))


def build_program(C: int):
    nc = bacc.Bacc("TRN2", debug=False)

    xsT = nc.dram_tensor("xsT", [128, KD, SC], BF16, kind="ExternalInput").ap()
    xgT = nc.dram_tensor("xgT", [128, KD, C], BF16, kind="ExternalInput").ap()
    sw1 = nc.dram_tensor("sw1", [NQ, 128, KD, 512], BF16, kind="ExternalInput").ap()
    sw2 = nc.dram_tensor("sw2", [128, KH, D], BF16, kind="ExternalInput").ap()
    w1 = nc.dram_tensor("w1", [NQ, 128, KD, 512], BF16, kind="ExternalInput").ap()
    w2 = nc.dram_tensor("w2", [128, KH, D], BF16, kind="ExternalInput").ap()
    b1s = nc.dram_tensor("b1s", [128, KH], F32, kind="ExternalInput").ap()
    b1e = nc.dram_tensor("b1e", [128, KH], F32, kind="ExternalInput").ap()
    outs = nc.dram_tensor("outs", [SC, D], F32, kind="ExternalOutput").ap()
    outg = nc.dram_tensor("outg", [C, D], F32, kind="ExternalOutput").ap()

    chunks = []
    c0 = 0
    while c0 < C:
        w = min(512, C - c0)
        chunks.append((c0, w))
        c0 += w

    with tile.TileContext(nc) as tc, ExitStack() as ctx:
        resp = ctx.enter_context(tc.tile_pool(name="resp", bufs=1))
        psp = ctx.enter_context(tc.tile_pool(name="psp", bufs=8, space="PSUM"))
        htp = resp
        otp = resp

        # Every DMA below is a contiguous [128, ...] slab (one descriptor
        # row per partition).  Only the first mm1 quarter's operands (split in
        # halves, ~1MB critical) are issued before the first matmuls; the bulk
        # loads are interleaved between mm1-quarter emissions via post_q so
        # they stream under compute instead of competing with the critical
        # pair for HBM bandwidth at t=0.
        sw1q = []
        sw1q.append(resp.tile([128, KD, 512], BF16, tag="sw1q0", name="sw1q0"))
        xsT_sb = resp.tile([128, KD, SC], BF16, tag="xsT")
        # first matmul needs only the k=0 slices (128KB each; per-queue DMA is
        # ~100-170GB/s with ~1.5us startup, so small first pieces matter)
        nc.sync.dma_start(out=sw1q[0][:, 0, :], in_=sw1[0][:, 0, :])
        nc.sync.dma_start(out=xsT_sb[:, 0, :], in_=xsT[:, 0, :])
        nc.sync.dma_start(out=sw1q[0][:, 1:4, :], in_=sw1[0][:, 1:4, :])
        nc.sync.dma_start(out=xsT_sb[:, 1:4, :], in_=xsT[:, 1:4, :])
        b1s_sb = resp.tile([128, KH], F32, tag="b1s")
        nc.sync.dma_start(out=b1s_sb, in_=b1s)
        nc.sync.dma_start(out=sw1q[0][:, 4:KD, :], in_=sw1[0][:, 4:KD, :])
        nc.sync.dma_start(out=xsT_sb[:, 4:KD, :], in_=xsT[:, 4:KD, :])
        for q in range(1, NQ):
            t = resp.tile([128, KD, 512], BF16, tag=f"sw1q{q}", name=f"sw1q{q}")
            sw1q.append(t)
        sw2_sb = resp.tile([128, KH, D], BF16, tag="sw2")
        xgT_sb = resp.tile([128, KD, C], BF16, tag="xgT")
        b1e_sb = resp.tile([128, KH], F32, tag="b1e")
        ew1q = [
            resp.tile([128, KD, 512], BF16, tag=f"ew1q{q}", name=f"ew1q_{q}")
            for q in range(NQ)
        ]
        ew2_sb = resp.tile([128, KH, D], BF16, tag="ew2")

        def _load_bulk():
            nc.sync.dma_start(out=sw2_sb, in_=sw2)
            nc.sync.dma_start(out=xgT_sb, in_=xgT)
            nc.sync.dma_start(out=b1e_sb, in_=b1e)
            for q in range(NQ):
                nc.sync.dma_start(out=ew1q[q], in_=w1[q])
            nc.sync.dma_start(out=ew2_sb, in_=w2)

        shared_post_q = {
            0: lambda: nc.sync.dma_start(out=sw1q[1], in_=sw1[1]),
            1: lambda: nc.sync.dma_start(out=sw1q[2], in_=sw1[2]),
            2: lambda: (nc.sync.dma_start(out=sw1q[3], in_=sw1[3]), _load_bulk()),
        }

        def emit_pass(xT_sb, c0, W, m_base, outdram, w1q, w2_sb, b1_sb, pi,
                      post_q=None, last=False):
            MTc = W // 128
            # mm1: hT[j] = gelu(w1.T @ xT + b1) in hid quarters of 4 psum banks
            hts = []
            for q in range(NQ):
                phs = [
                    psp.tile([128, W], F32, tag="ps", name=f"ph{pi}_{q}_{mh}")
                    for mh in range(4)
                ]
                for k in range(KD):
                    for mh in range(4):
                        nc.tensor.matmul(
                            phs[mh],
                            w1q[q][:, k, mh * 128 : (mh + 1) * 128],
                            xT_sb[:, k, c0 : c0 + W],
                            start=(k == 0),
                            stop=(k == KD - 1),
                        )
                if post_q and q in post_q:
                    post_q[q]()
                for mh in range(4):
                    j = q * 4 + mh
                    ht = htp.tile([128, 512], BF16, tag=f"ht{j}", name=f"ht{pi}_{j}")
                    nc.scalar.activation(
                        ht[:, :W], phs[mh][:], AF.Gelu, bias=b1_sb[:, j : j + 1]
                    )
                    hts.append(ht)

            # mm2: out[mt] = sum_k hT[k][:, mt].T @ w2[k]; m-tile-outer so each
            # m-tile evicts (DVE || ACT copy halves) under the next one's MMs.
            ov = outdram.rearrange("(m p) d -> p m d", p=128)
            for mt in range(MTc):
                pon = [
                    psp.tile([128, 512], F32, tag="ps", name=f"po{pi}_{mt}_{n}")
                    for n in range(2)
                ]
                for k in range(KH):
                    for n in range(2):
                        nc.tensor.matmul(
                            pon[n],
                            hts[k][:, mt * 128 : (mt + 1) * 128],
                            w2_sb[:, k, n * 512 : (n + 1) * 512],
                            start=(k == 0),
                            stop=(k == KH - 1),
                        )
                ot = otp.tile([128, D], F32, tag="ot", bufs=4, name=f"ot{pi}_{mt}")
                nc.vector.tensor_copy(ot[:, 0:512], pon[0][:])
                if last and mt == MTc - 1:
                    # final m-tile: start the n0-half store while the ACT copy
                    # of the n1-half is still draining, shortening the tail
                    nc.sync.dma_start(
                        out=ov[:, m_base + mt, 0:512], in_=ot[:, 0:512]
                    )
                    nc.scalar.copy(ot[:, 512:1024], pon[1][:])
                    nc.sync.dma_start(
                        out=ov[:, m_base + mt, 512:1024], in_=ot[:, 512:1024]
                    )
                else:
                    nc.scalar.copy(ot[:, 512:1024], pon[1][:])
                    nc.sync.dma_start(out=ov[:, m_base + mt, :], in_=ot)

        emit_pass(xsT_sb, 0, SC, 0, outs, sw1q, sw2_sb, b1s_sb, 0,
                  post_q=shared_post_q)
        for ci, (c0, w) in enumerate(chunks):
            emit_pass(xgT_sb, c0, w, c0 // 128, outg, ew1q, ew2_sb, b1e_sb,
                      1 + ci, last=(ci == len(chunks) - 1))

    nc.compile()
    return nc


_programs: dict = {}
LAST_RESULTS = None


def _get_program(C: int):
    if C not in _programs:
        _programs[C] = build_program(C)
    return _programs[C]


def _route_jax(flat, router_w, router_b):
    """Replicate reference router bit-for-bit (same jax CPU ops)."""
    import jax
    import jax.numpy as jnp

    cpu = jax.devices("cpu")[0]
    with jax.default_device(cpu):
        probs = jax.nn.softmax(
            jnp.asarray(flat) @ jnp.asarray(router_w) + jnp.asarray(router_b), axis=-1
        )
        top_w, top_i = jax.lax.top_k(probs, TOP_K)
        top_w = top_w / jnp.sum(top_w, axis=-1, keepdims=True)
        return np.asarray(top_w), np.asarray(top_i)


def _route_np(flat, router_w, router_b):
    logits = (
        flat.astype(np.float64) @ router_w.astype(np.float64)
        + router_b.astype(np.float64)
    )
    ar = np.arange(T)
    i1 = np.argmax(logits, 1)
    l1 = logits[ar, i1]
    lm = logits.copy()
    lm[ar, i1] = -np.inf
    i2 = np.argmax(lm, 1)
    l2 = lm[ar, i2]
    wa = 1.0 / (1.0 + np.exp(l2 - l1))
    top_w = np.stack([wa, 1.0 - wa], 1).astype(np.float32)
    top_i = np.stack([i1, i2], 1).astype(np.int32)
    return top_w, top_i


def kernel(x, router_w, router_b, sw1, sb1, sw2, sb2, ew1, eb1, ew2, eb2):
    global LAST_RESULTS
    x = np.asarray(x, np.float32)
    flat = np.ascontiguousarray(x.reshape(T, D))
    rw = np.ascontiguousarray(np.asarray(router_w, np.float32))
    rb = np.asarray(router_b, np.float32).reshape(E)
    try:
        top_w, top_i = _route_jax(flat, rw, rb)
    except Exception:
        top_w, top_i = _route_np(flat, rw, rb)
    i1 = top_i[:, 0].astype(np.int64)
    i2 = top_i[:, 1].astype(np.int64)

    rows_l, wgt_l = [], []
    for e in range(E):
        sel1 = i1 == e
        rows = np.nonzero(sel1 | (i2 == e))[0]
        wgt = np.where(sel1[rows], top_w[rows, 0], top_w[rows, 1]).astype(np.float32)
        rows_l.append(rows)
        wgt_l.append(wgt)
    maxc = max(len(r) for r in rows_l)
    # Device capacity is the perfectly balanced T*K/E; the few overflow
    # assignments beyond it (load imbalance tail) are computed exactly on the
    # host, keeping every core's padded pass the same minimal size.
    CAP = T * TOP_K // E
    C = max(128, min(-(-maxc // 128) * 128, CAP))

    nc = _get_program(C)

    xq = flat.astype(BF)
    sw1p = _pack_w1(np.asarray(sw1, np.float32))
    sw2p = _pack_w2(np.asarray(sw2, np.float32))
    ew1f = np.asarray(ew1, np.float32)
    ew2f = np.asarray(ew2, np.float32)
    b1s_arr = np.ascontiguousarray(np.asarray(sb1, np.float32).reshape(KH, 128).T)
    eb1f = np.asarray(eb1, np.float32)

    in_maps = []
    for c in range(NCORES):
        rows = rows_l[c][:C]
        in_maps.append(
            {
                "xsT": _pack_xT(xq[c * SC : (c + 1) * SC], SC),
                "xgT": _pack_xT(xq[rows], C),
                "sw1": sw1p,
                "sw2": sw2p,
                "w1": _pack_w1(ew1f[c]),
                "w2": _pack_w2(ew2f[c]),
                "b1s": b1s_arr,
                "b1e": np.ascontiguousarray(eb1f[c].reshape(KH, 128).T),
            }
        )

    def _spot_check(res) -> bool:
        """Recompute a few shared-expert rows on host; catches a wedged
        device returning stale/garbage buffers (seen after NRT errors)."""
        sw1f = np.asarray(sw1, np.float32)
        sw2f = np.asarray(sw2, np.float32)
        sb1f = np.asarray(sb1, np.float32).reshape(H)
        for c in (0, NCORES - 1):
            got = res.results[c]["outs"]
            if not np.all(np.isfinite(got)):
                return False
            t = c * SC + 7
            ref = _gelu_exact(flat[t] @ sw1f + sb1f) @ sw2f
            scale = max(1.0, float(np.abs(ref).max()))
            if float(np.abs(got[7] - ref).max()) > 1e-2 * scale:
                return False
        return True

    res = None
    for attempt in range(4):
        try:
            res = run_bass_kernel_spmd(nc, in_maps, core_ids=list(range(NCORES)))
            if _spot_check(res):
                break
            res = None
        except Exception:
            if attempt == 3:
                raise
        import time as _time

        _time.sleep(5)  # transient device errors recover on retry
    if res is None:
        raise RuntimeError("device returned inconsistent results on all attempts")
    LAST_RESULTS = res

    out = np.ascontiguousarray(
        np.concatenate([res.results[c]["outs"] for c in range(NCORES)], axis=0),
        dtype=np.float32,
    )
    for e in range(E):
        rows = rows_l[e][:C]
        if len(rows):
            out[rows] += (
                wgt_l[e][: len(rows), None] * res.results[e]["outg"][: len(rows)]
            )
        over = rows_l[e][C:]
        if len(over):
            # exact fp32 host compute for capacity-overflow assignments
            z = flat[over] @ np.asarray(ew1[e], np.float32) + eb1f[e]
            y = _gelu_exact(z) @ np.asarray(ew2[e], np.float32)
            out[over] += wgt_l[e][C:, None] * y

    sb2f = np.asarray(sb2, np.float32).reshape(D)
    if sb2f.any():
        out += sb2f[None, :]
    eb2f = np.asarray(eb2, np.float32)
    if eb2f.any():
        comb = np.zeros((T, E), np.float32)
        comb[np.arange(T), i1] = top_w[:, 0]
        comb[np.arange(T), i2] = top_w[:, 1]
        out += comb @ eb2f
    return out.reshape(B, S, D)
